# revision 2
# baseline (speedup 1.0000x reference)
"""Trainium2 Bass kernel for the 4-layer sum/product circuit
(nn_KnowledgeLayer): h = enc(x); h = h[idx0].prod(1); h = h[idx1].sum(1);
h = h[idx2].prod(1); h = h[idx3].sum(1).

Strategy (per core, batch sharded 8 x 128 columns, batch on PARTITIONS):
  * Host composes the four index maps into ONE fully-expanded gather from
    the 4098-entry enc table (enc = [x | 1-x | 0 | 1]), producing two leaf
    streams A, B of 32768 indices ordered so each circuit layer reduces
    adjacent free-axis pairs: h0 = gA*gB, h1 = h0[0::2]+h0[1::2],
    h2 = h1[0::2]*h1[1::2], h3 = h2[0::2]+h2[1::2].
  * Device keeps encT as [128 batch, 4098] in SBUF and uses GPSIMD
    ap_gather (free-axis gather, on-chip) instead of SWDGE dma_gather —
    no per-row DMA descriptors at all.  DVE does the elementwise layers,
    PE transposes h3 back to [rows, batch] via identity matmul, ACT
    drains PSUM to SBUF, and plain DMAs move xT in / out rows out.

The bass program is identical for all 8 cores (pure SPMD); per-core data
(xT batch slice) is supplied via in_maps.
"""

import numpy as np

N_VARS = 2048
BATCH = 1024
NCORES = 8
BSLICE = BATCH // NCORES          # 128
TABLE = 2 * N_VARS + 2            # 4098
NIDX = 32768                      # leaves per stream
NOUT = 4096

NCHUNK = 16
CH = NIDX // NCHUNK               # leaves per stream per chunk (2048)
OUTC = CH // 8                    # output rows per chunk (256)


# ----------------------------------------------------------------------------
# host-side index preparation
# ----------------------------------------------------------------------------

def _remap(e):
    """reference enc row -> our encT column."""
    e = e.astype(np.int64)
    out = np.empty_like(e)
    out[e == 0] = 2 * N_VARS
    out[e == 1] = 2 * N_VARS + 1
    even = (e >= 2) & (e % 2 == 0)
    out[even] = (e[even] - 2) // 2
    odd = (e >= 3) & (e % 2 == 1)
    out[odd] = N_VARS + (e[odd] - 3) // 2
    return out


def _compose_indices(idx0, idx1, idx2, idx3):
    n2 = idx3.reshape(-1)                 # [8192]  h2 nodes, pair-adjacent
    n1 = idx2[n2].reshape(-1)             # [16384] h1 nodes
    n0 = idx1[n1].reshape(-1)             # [32768] h0 nodes
    ab = idx0[n0]                         # [32768, 2] enc rows
    return _remap(ab[:, 0]), _remap(ab[:, 1])


def _wrap_idx(stream_idx):
    """ap_gather index layout: stream elem k lives at partition k%16,
    column k//16, replicated across the 8 gpsimd cores (16 partitions each)."""
    w = stream_idx.reshape(-1, 16).T.astype(np.int16)   # [16, NIDX//16]
    return np.ascontiguousarray(np.tile(w, (8, 1)))     # [128, NIDX//16]


# ----------------------------------------------------------------------------
# bass program (built once per repeat count, cached)
# ----------------------------------------------------------------------------

_CACHED = {}


def _build_program(repeat=1):
    import contextlib
    import concourse.bacc as bacc
    import concourse.mybir as mybir
    from concourse.tile import TileContext

    f32 = mybir.dt.float32
    i16 = mybir.dt.int16

    nc = bacc.Bacc("TRN2", target_bir_lowering=False, debug=False)

    xT = nc.dram_tensor("xT", [BSLICE, N_VARS], f32, kind="ExternalInput")
    ia = nc.dram_tensor("ia", [128, NIDX // 16], i16, kind="ExternalInput")
    ib = nc.dram_tensor("ib", [128, NIDX // 16], i16, kind="ExternalInput")
    ident = nc.dram_tensor("ident", [128, 128], f32, kind="ExternalInput")
    out = nc.dram_tensor("out", [NOUT, BSLICE], f32, kind="ExternalOutput")

    with TileContext(nc) as tc:
        with tc.tile_pool(name="setup", bufs=1) as sp, \
             tc.tile_pool(name="gather", bufs=3) as gp, \
             tc.tile_pool(name="mid", bufs=2) as mp, \
             tc.tile_pool(name="stage", bufs=3) as stp, \
             tc.tile_pool(name="psum", bufs=4, space="PSUM") as pp:

            # ---- build encT = [x | 1-x | 0 | 1] on batch partitions ----
            encT = sp.tile([128, TABLE], f32, tag="encT")
            nc.sync.dma_start(out=encT[:, 0:N_VARS], in_=xT[:, :])
            nc.vector.tensor_scalar(
                out=encT[:, N_VARS:2 * N_VARS], in0=encT[:, 0:N_VARS],
                scalar1=-1.0, scalar2=1.0,
                op0=mybir.AluOpType.mult, op1=mybir.AluOpType.add)
            nc.vector.memset(encT[:, 2 * N_VARS:2 * N_VARS + 1], 0.0)
            nc.vector.memset(encT[:, 2 * N_VARS + 1:2 * N_VARS + 2], 1.0)

            # ---- index streams + transpose identity ----
            idA = sp.tile([128, NIDX // 16], i16, tag="idA")
            idB = sp.tile([128, NIDX // 16], i16, tag="idB")
            nc.sync.dma_start(out=idA[:, :], in_=ia[:, :])
            nc.sync.dma_start(out=idB[:, :], in_=ib[:, :])
            idn = sp.tile([128, 128], f32, tag="idn")
            nc.sync.dma_start(out=idn[:, :], in_=ident[:, :])

            # ---- main chunk loop ----
            loop_ctx = tc.For_i(0, repeat, 1) if repeat > 1 else contextlib.nullcontext()
            with loop_ctx:
                for c in range(NCHUNK):
                    ga = gp.tile([128, CH], f32, tag="ga")
                    gb = gp.tile([128, CH], f32, tag="gb")
                    cols = slice(c * (CH // 16), (c + 1) * (CH // 16))
                    nc.gpsimd.ap_gather(
                        out_ap=ga[:, :], in_ap=encT[:, :],
                        idxs_ap=idA[:, cols],
                        channels=128, num_elems=TABLE, d=1, num_idxs=CH)
                    nc.gpsimd.ap_gather(
                        out_ap=gb[:, :], in_ap=encT[:, :],
                        idxs_ap=idB[:, cols],
                        channels=128, num_elems=TABLE, d=1, num_idxs=CH)

                    h0 = mp.tile([128, CH], f32, tag="h0")
                    nc.vector.tensor_mul(h0[:, :], ga[:, :], gb[:, :])
                    h1 = mp.tile([128, CH // 2], f32, tag="h1")
                    nc.vector.tensor_add(h1[:, :], h0[:, 0:CH:2], h0[:, 1:CH:2])
                    h2 = mp.tile([128, CH // 4], f32, tag="h2")
                    nc.vector.tensor_mul(h2[:, :], h1[:, 0:CH // 2:2], h1[:, 1:CH // 2:2])
                    h3 = mp.tile([128, OUTC], f32, tag="h3")
                    nc.vector.tensor_add(h3[:, :], h2[:, 0:CH // 4:2], h2[:, 1:CH // 4:2])

                    # transpose [128 batch, OUTC rows] -> [OUTC rows, 128 batch]
                    ps = pp.tile([128, OUTC // 128, 128], f32, tag="ps")
                    for t in range(OUTC // 128):
                        nc.tensor.transpose(
                            ps[:, t, :], h3[:, t * 128:(t + 1) * 128], idn[:, :])
                    st = stp.tile([128, OUTC // 128, 128], f32, tag="st")
                    nc.scalar.copy(out=st[:, :, :], in_=ps[:, :, :])
                    nc.sync.dma_start(
                        out=out[c * OUTC:(c + 1) * OUTC, :]
                            .rearrange("(t p) f -> p t f", p=128),
                        in_=st[:, :, :])

    nc.compile()
    return nc


def _get_program(repeat=1):
    key = ("nc", repeat)
    if key not in _CACHED:
        _CACHED[key] = _build_program(repeat)
    return _CACHED[key]


# ----------------------------------------------------------------------------
# public entry point
# ----------------------------------------------------------------------------

def _prep_inputs(x, idx0, idx1, idx2, idx3):
    x = np.ascontiguousarray(np.asarray(x, dtype=np.float32))
    sA, sB = _compose_indices(
        np.asarray(idx0), np.asarray(idx1), np.asarray(idx2), np.asarray(idx3))
    wa, wb = _wrap_idx(sA), _wrap_idx(sB)
    ident = np.ascontiguousarray(np.eye(128, dtype=np.float32))
    in_maps = []
    for c in range(NCORES):
        xs = np.ascontiguousarray(x[:, c * BSLICE:(c + 1) * BSLICE].T)
        in_maps.append({"xT": xs, "ia": wa, "ib": wb, "ident": ident})
    return in_maps


def kernel(x, idx0, idx1, idx2, idx3, _repeat=1):
    from concourse.bass_utils import run_bass_kernel_spmd

    in_maps = _prep_inputs(x, idx0, idx1, idx2, idx3)
    nc = _get_program(_repeat)
    res = run_bass_kernel_spmd(nc, in_maps, core_ids=list(range(NCORES)))
    outs = [res.results[c]["out"] for c in range(NCORES)]
    return np.concatenate(outs, axis=1)


# revision 11
# speedup vs baseline: 1.4709x; 1.4709x over previous
"""Trainium2 Bass kernel for the 4-layer sum/product circuit
(nn_KnowledgeLayer): h = enc(x); h = h[idx0].prod(1); h = h[idx1].sum(1);
h = h[idx2].prod(1); h = h[idx3].sum(1).

Strategy (per core, batch sharded 8 x 128 columns, batch on PARTITIONS):
  * Host composes the four index maps into ONE fully-expanded gather from
    the 4098-entry enc table (enc = [x | 1-x | 0 | 1]).  Leaves are
    ordered "fold-halves" per chunk: the two operands of every reduction
    sit exactly half-a-chunk apart, so every circuit layer is a single
    CONTIGUOUS DVE op: h0 = g[:, :n]*g[:, n:], h1 = h0[:, :n/2]+h0[:, n/2:], ...
  * Device keeps encT as [128 batch, 4098] in SBUF and gathers along the
    FREE axis on the Pool engine (native InstIndirectCopy, or the gpsimd
    ap_gather ucode as fallback) — no per-row DMA descriptors at all.
  * PE transposes h3 back to [rows, batch] via identity matmul, ACT
    drains PSUM to SBUF, DMA writes a tile-major [128, 32, 128] DRAM
    layout (per-partition contiguous => fat descriptors); the host
    unshard step permutes tiles back to row-major.

The bass program is identical for all 8 cores (pure SPMD); per-core data
(xT batch slice) is supplied via in_maps.
"""

import numpy as np

N_VARS = 2048
BATCH = 1024
NCORES = 8
BSLICE = BATCH // NCORES          # 128
TABLE = 2 * N_VARS + 2            # 4098
NIDX = 32768                      # leaves per operand stream
NOUT = 4096

NCHUNK = 8                        # default chunking of the main loop
IMPL = "apg"                      # ap_gather ucode (indirect_copy hits ISA limits and is 3-idx/req anyway)


# ----------------------------------------------------------------------------
# host-side index preparation
# ----------------------------------------------------------------------------

def _remap(e):
    """reference enc row -> our encT column."""
    e = e.astype(np.int64)
    out = np.empty_like(e)
    out[e == 0] = 2 * N_VARS
    out[e == 1] = 2 * N_VARS + 1
    even = (e >= 2) & (e % 2 == 0)
    out[even] = (e[even] - 2) // 2
    odd = (e >= 3) & (e % 2 == 1)
    out[odd] = N_VARS + (e[odd] - 3) // 2
    return out


def _compose_stream(idx0, idx1, idx2, idx3, nchunk):
    """Fold-halves leaf ordering: one merged stream, 16*outc leaves/chunk."""
    outc = NOUT // nchunk
    parts = []
    for c in range(nchunk):
        sl = slice(c * outc, (c + 1) * outc)
        n2 = np.concatenate([idx3[sl, 0], idx3[sl, 1]])
        n1 = np.concatenate([idx2[n2, 0], idx2[n2, 1]])
        n0 = np.concatenate([idx1[n1, 0], idx1[n1, 1]])
        lv = np.concatenate([idx0[n0, 0], idx0[n0, 1]])
        parts.append(_remap(lv))
    return np.concatenate(parts)          # [2 * NIDX]


def _wrap_idx(stream_idx, dtype):
    """Pool-engine index layout: stream elem k lives at partition k%16,
    column k//16, replicated across the 8 groups of 16 partitions."""
    w = stream_idx.reshape(-1, 16).T.astype(dtype)      # [16, len/16]
    return np.ascontiguousarray(np.tile(w, (8, 1)))     # [128, len/16]


# ----------------------------------------------------------------------------
# bass program (built once per config, cached)
# ----------------------------------------------------------------------------

_CACHED = {}


def _build_program(repeat=1, mode="full", nchunk=NCHUNK, impl=IMPL):
    import contextlib
    import concourse.bacc as bacc
    import concourse.mybir as mybir
    from concourse.tile import TileContext

    f32 = mybir.dt.float32
    idt = mybir.dt.uint16 if impl == "ic" else mybir.dt.int16
    ch = 2 * NIDX // nchunk           # merged leaves per chunk
    outc = ch // 16                   # output rows per chunk
    ntile = outc // 128               # 128-row transpose tiles per chunk

    do_gather = mode in ("full", "gather", "gtiny")
    do_dve = mode in ("full", "dve")
    do_out = mode in ("full", "dve", "out")
    if mode == "gtiny":               # per-call overhead probe: tiny gathers
        ch = 64

    nc = bacc.Bacc("TRN2", target_bir_lowering=False, debug=False)

    xT = nc.dram_tensor("xT", [BSLICE, N_VARS], f32, kind="ExternalInput")
    si = nc.dram_tensor("si", [128, 2 * NIDX // 16], idt, kind="ExternalInput")
    ident = nc.dram_tensor("ident", [128, 128], f32, kind="ExternalInput")
    # tile-major output: [partition, tile, batch]; host permutes to row-major
    out = nc.dram_tensor("out", [128, NOUT // 128, 128], f32,
                         kind="ExternalOutput")

    psum_bufs = 2 if ntile >= 4 else 4
    with TileContext(nc) as tc:
        with tc.tile_pool(name="setup", bufs=1) as sp, \
             tc.tile_pool(name="gather", bufs=2) as gp, \
             tc.tile_pool(name="stage", bufs=3) as stp, \
             tc.tile_pool(name="psum", bufs=psum_bufs, space="PSUM") as pp:

            # ---- build encT = [x | 1-x | 0 | 1] on batch partitions ----
            encT = sp.tile([128, TABLE], f32, tag="encT")
            nc.sync.dma_start(out=encT[:, 0:N_VARS], in_=xT[:, :])
            nc.vector.tensor_scalar(
                out=encT[:, N_VARS:2 * N_VARS], in0=encT[:, 0:N_VARS],
                scalar1=-1.0, scalar2=1.0,
                op0=mybir.AluOpType.mult, op1=mybir.AluOpType.add)
            nc.vector.memset(encT[:, 2 * N_VARS:2 * N_VARS + 1], 0.0)
            nc.vector.memset(encT[:, 2 * N_VARS + 1:2 * N_VARS + 2], 1.0)

            # ---- index stream + transpose identity ----
            idS = sp.tile([128, 2 * NIDX // 16], idt, tag="idS")
            ncols = 2 * NIDX // 16
            for c in range(nchunk):
                csl = slice(c * ncols // nchunk, (c + 1) * ncols // nchunk)
                nc.sync.dma_start(out=idS[:, csl], in_=si[:, csl])
            idn = sp.tile([128, 128], f32, tag="idn")
            nc.sync.dma_start(out=idn[:, :], in_=ident[:, :])

            # ablation placeholders for skipped stages
            if not do_gather:
                gf = sp.tile([128, ch], f32, tag="gf")
                nc.scalar.memzero(gf[:, :])
            if not do_dve and do_out:
                h3f = sp.tile([128, outc], f32, tag="h3f")
                nc.scalar.memzero(h3f[:, :])

            # ---- main chunk loop ----
            loop_ctx = tc.For_i(0, repeat, 1) if repeat > 1 else contextlib.nullcontext()
            with loop_ctx:
                for c in range(nchunk):
                    if do_gather:
                        g = gp.tile([128, ch], f32, tag="g")
                        cols = slice(c * (ch // 16), (c + 1) * (ch // 16))
                        if impl == "ic":
                            nc.gpsimd.indirect_copy(
                                out=g[:, :], data=encT[:, :],
                                idxs=idS[:, cols],
                                i_know_ap_gather_is_preferred=True)
                        else:
                            nc.gpsimd.ap_gather(
                                out_ap=g[:, :], in_ap=encT[:, :],
                                idxs_ap=idS[:, cols],
                                channels=128, num_elems=TABLE, d=1,
                                num_idxs=ch)
                    else:
                        g = gf

                    if do_dve:
                        # fold-halves reductions, in place in g's prefix
                        nc.vector.tensor_mul(
                            g[:, 0:ch // 2], g[:, 0:ch // 2], g[:, ch // 2:ch])
                        nc.vector.tensor_add(
                            g[:, 0:ch // 4], g[:, 0:ch // 4], g[:, ch // 4:ch // 2])
                        nc.vector.tensor_mul(
                            g[:, 0:ch // 8], g[:, 0:ch // 8], g[:, ch // 8:ch // 4])
                        nc.vector.tensor_add(
                            g[:, 0:outc], g[:, 0:outc], g[:, outc:2 * outc])
                        h3 = g
                    elif do_out:
                        h3 = h3f

                    if do_out:
                        # transpose [128 batch, outc rows] -> [rows, 128]
                        ps = pp.tile([128, ntile, 128], f32, tag="ps")
                        for t in range(ntile):
                            nc.tensor.transpose(
                                ps[:, t, :], h3[:, t * 128:(t + 1) * 128],
                                idn[:, :])
                        st = stp.tile([128, ntile, 128], f32, tag="st")
                        nc.scalar.copy(out=st[:, :, :], in_=ps[:, :, :])
                        nc.sync.dma_start(
                            out=out[:, c * ntile:(c + 1) * ntile, :],
                            in_=st[:, :, :])

    nc.compile()
    return nc


def _get_program(repeat=1, mode="full", nchunk=NCHUNK, impl=IMPL):
    key = ("nc", repeat, mode, nchunk, impl)
    if key not in _CACHED:
        _CACHED[key] = _build_program(repeat, mode, nchunk, impl)
    return _CACHED[key]


# ----------------------------------------------------------------------------
# public entry point
# ----------------------------------------------------------------------------

def _prep_inputs(x, idx0, idx1, idx2, idx3, nchunk=NCHUNK, impl=IMPL):
    x = np.ascontiguousarray(np.asarray(x, dtype=np.float32))
    stream = _compose_stream(
        np.asarray(idx0), np.asarray(idx1), np.asarray(idx2), np.asarray(idx3),
        nchunk)
    ws = _wrap_idx(stream, np.uint16 if impl == "ic" else np.int16)
    ident = np.ascontiguousarray(np.eye(128, dtype=np.float32))
    in_maps = []
    for c in range(NCORES):
        xs = np.ascontiguousarray(x[:, c * BSLICE:(c + 1) * BSLICE].T)
        in_maps.append({"xT": xs, "si": ws, "ident": ident})
    return in_maps


def _unshard(res):
    outs = []
    for c in range(NCORES):
        o = res.results[c]["out"]                 # [128, 32, 128] tile-major
        outs.append(o.transpose(1, 0, 2).reshape(NOUT, BSLICE))
    return np.concatenate(outs, axis=1)


def kernel(x, idx0, idx1, idx2, idx3, _repeat=1):
    from concourse.bass_utils import run_bass_kernel_spmd

    in_maps = _prep_inputs(x, idx0, idx1, idx2, idx3)
    nc = _get_program(_repeat)
    res = run_bass_kernel_spmd(nc, in_maps, core_ids=list(range(NCORES)))
    return _unshard(res)


# revision 14
# speedup vs baseline: 1.6744x; 1.1383x over previous
"""Trainium2 Bass kernel for the 4-layer sum/product circuit
(nn_KnowledgeLayer): h = enc(x); h = h[idx0].prod(1); h = h[idx1].sum(1);
h = h[idx2].prod(1); h = h[idx3].sum(1).

Strategy (per core, batch sharded 8 x 128 columns, batch on PARTITIONS):
  * Host composes the four index maps into ONE fully-expanded gather from
    the 4098-entry enc table (enc = [x | 1-x | 0 | 1]).  Leaves are
    ordered "fold-halves" per chunk: the two operands of every reduction
    sit exactly half-a-chunk apart, so every circuit layer is a single
    CONTIGUOUS DVE op: h0 = g[:, :n]*g[:, n:], h1 = h0[:, :n/2]+h0[:, n/2:], ...
  * Device keeps encT as [128 batch, 4098] in SBUF and gathers along the
    FREE axis with gpsimd ap_gather (8 Q7 cores x 16 partitions each) —
    no per-row DMA descriptors at all.  ~23ns/index + ~10us/call is the
    cayman RD_CMD floor (102 cyc / 4 idx, ReadOverlap=0).
  * PE transposes h3 back to [rows, batch] via identity matmul, ACT
    drains PSUM to SBUF, DMA writes a tile-major [128, 32, 128] DRAM
    layout (per-partition contiguous => fat descriptors); the host
    unshard step permutes tiles back to row-major.

The bass program is identical for all 8 cores (pure SPMD); per-core data
(xT batch slice) is supplied via in_maps.
"""

import numpy as np

N_VARS = 2048
BATCH = 1024
NCORES = 8
BSLICE = BATCH // NCORES          # 128
TABLE = 2 * N_VARS + 2            # 4098
NIDX = 32768                      # leaves per operand stream
NOUT = 4096

NCHUNK = 8                        # default chunking of the main loop
IMPL = "apg"                      # ap_gather ucode (indirect_copy hits ISA limits and is 3-idx/req anyway)


# ----------------------------------------------------------------------------
# host-side index preparation
# ----------------------------------------------------------------------------

def _remap(e):
    """reference enc row -> our encT column."""
    e = e.astype(np.int64)
    out = np.empty_like(e)
    out[e == 0] = 2 * N_VARS
    out[e == 1] = 2 * N_VARS + 1
    even = (e >= 2) & (e % 2 == 0)
    out[even] = (e[even] - 2) // 2
    odd = (e >= 3) & (e % 2 == 1)
    out[odd] = N_VARS + (e[odd] - 3) // 2
    return out


def _compose_stream(idx0, idx1, idx2, idx3, nchunk):
    """Fold-halves leaf ordering: one merged stream, 16*outc leaves/chunk."""
    outc = NOUT // nchunk
    parts = []
    for c in range(nchunk):
        sl = slice(c * outc, (c + 1) * outc)
        n2 = np.concatenate([idx3[sl, 0], idx3[sl, 1]])
        n1 = np.concatenate([idx2[n2, 0], idx2[n2, 1]])
        n0 = np.concatenate([idx1[n1, 0], idx1[n1, 1]])
        lv = np.concatenate([idx0[n0, 0], idx0[n0, 1]])
        parts.append(_remap(lv))
    return np.concatenate(parts)          # [2 * NIDX]


def _wrap_idx(stream_idx, dtype):
    """Pool-engine index layout: stream elem k lives at partition k%16,
    column k//16, replicated across the 8 groups of 16 partitions."""
    w = stream_idx.reshape(-1, 16).T.astype(dtype)      # [16, len/16]
    return np.ascontiguousarray(np.tile(w, (8, 1)))     # [128, len/16]


# ----------------------------------------------------------------------------
# bass program (built once per config, cached)
# ----------------------------------------------------------------------------

_CACHED = {}


def _build_program(repeat=1, mode="full", nchunk=NCHUNK, impl=IMPL):
    import contextlib
    import concourse.bacc as bacc
    import concourse.mybir as mybir
    from concourse.tile import TileContext

    f32 = mybir.dt.float32
    idt = mybir.dt.uint16 if impl == "ic" else mybir.dt.int16
    ch = 2 * NIDX // nchunk           # merged leaves per chunk
    outc = ch // 16                   # output rows per chunk
    ntile = outc // 128               # 128-row transpose tiles per chunk

    do_gather = mode in ("full", "gather", "gtiny")
    do_dve = mode in ("full", "dve")
    do_out = mode in ("full", "dve", "out")
    if mode == "gtiny":               # per-call overhead probe: tiny gathers
        ch = 64

    nc = bacc.Bacc("TRN2", target_bir_lowering=False, debug=False)

    xT = nc.dram_tensor("xT", [BSLICE, N_VARS], f32, kind="ExternalInput")
    si = nc.dram_tensor("si", [128, 2 * NIDX // 16], idt, kind="ExternalInput")
    ident = nc.dram_tensor("ident", [128, 128], f32, kind="ExternalInput")
    # tile-major output: [partition, tile, batch]; host permutes to row-major
    out = nc.dram_tensor("out", [128, NOUT // 128, 128], f32,
                         kind="ExternalOutput")

    psum_bufs = 2 if ntile >= 4 else 4
    with TileContext(nc) as tc:
        with tc.tile_pool(name="setup", bufs=1) as sp, \
             tc.tile_pool(name="gather", bufs=3) as gp, \
             tc.tile_pool(name="stage", bufs=3) as stp, \
             tc.tile_pool(name="psum", bufs=psum_bufs, space="PSUM") as pp:

            # ---- build encT = [x | 1-x | 0 | 1] on batch partitions ----
            encT = sp.tile([128, TABLE], f32, tag="encT")
            nc.sync.dma_start(out=encT[:, 0:N_VARS], in_=xT[:, :])
            nc.vector.tensor_scalar(
                out=encT[:, N_VARS:2 * N_VARS], in0=encT[:, 0:N_VARS],
                scalar1=-1.0, scalar2=1.0,
                op0=mybir.AluOpType.mult, op1=mybir.AluOpType.add)
            nc.vector.memset(encT[:, 2 * N_VARS:2 * N_VARS + 1], 0.0)
            nc.vector.memset(encT[:, 2 * N_VARS + 1:2 * N_VARS + 2], 1.0)

            # ---- index stream + transpose identity ----
            idS = sp.tile([128, 2 * NIDX // 16], idt, tag="idS")
            ncols = 2 * NIDX // 16
            for c in range(nchunk):
                csl = slice(c * ncols // nchunk, (c + 1) * ncols // nchunk)
                nc.sync.dma_start(out=idS[:, csl], in_=si[:, csl])
            idn = sp.tile([128, 128], f32, tag="idn")
            nc.sync.dma_start(out=idn[:, :], in_=ident[:, :])

            # ablation placeholders for skipped stages
            if not do_gather:
                gf = sp.tile([128, ch], f32, tag="gf")
                nc.scalar.memzero(gf[:, :])
            if not do_dve and do_out:
                h3f = sp.tile([128, outc], f32, tag="h3f")
                nc.scalar.memzero(h3f[:, :])

            # ---- main chunk loop ----
            loop_ctx = tc.For_i(0, repeat, 1) if repeat > 1 else contextlib.nullcontext()
            with loop_ctx:
                for c in range(nchunk):
                    if do_gather:
                        g = gp.tile([128, ch], f32, tag="g")
                        cols = slice(c * (ch // 16), (c + 1) * (ch // 16))
                        if impl == "ic":
                            nc.gpsimd.indirect_copy(
                                out=g[:, :], data=encT[:, :],
                                idxs=idS[:, cols],
                                i_know_ap_gather_is_preferred=True)
                        else:
                            nc.gpsimd.ap_gather(
                                out_ap=g[:, :], in_ap=encT[:, :],
                                idxs_ap=idS[:, cols],
                                channels=128, num_elems=TABLE, d=1,
                                num_idxs=ch)
                    else:
                        g = gf

                    if do_dve:
                        # fold-halves reductions, in place in g's prefix
                        nc.vector.tensor_mul(
                            g[:, 0:ch // 2], g[:, 0:ch // 2], g[:, ch // 2:ch])
                        nc.vector.tensor_add(
                            g[:, 0:ch // 4], g[:, 0:ch // 4], g[:, ch // 4:ch // 2])
                        nc.vector.tensor_mul(
                            g[:, 0:ch // 8], g[:, 0:ch // 8], g[:, ch // 8:ch // 4])
                        nc.vector.tensor_add(
                            g[:, 0:outc], g[:, 0:outc], g[:, outc:2 * outc])
                        h3 = g
                    elif do_out:
                        h3 = h3f

                    if do_out:
                        # transpose [128 batch, outc rows] -> [rows, 128]
                        ps = pp.tile([128, ntile, 128], f32, tag="ps")
                        for t in range(ntile):
                            nc.tensor.transpose(
                                ps[:, t, :], h3[:, t * 128:(t + 1) * 128],
                                idn[:, :])
                        st = stp.tile([128, ntile, 128], f32, tag="st")
                        nc.scalar.copy(out=st[:, :, :], in_=ps[:, :, :])
                        nc.sync.dma_start(
                            out=out[:, c * ntile:(c + 1) * ntile, :],
                            in_=st[:, :, :])

    nc.compile()
    return nc


def _get_program(repeat=1, mode="full", nchunk=NCHUNK, impl=IMPL):
    key = ("nc", repeat, mode, nchunk, impl)
    if key not in _CACHED:
        _CACHED[key] = _build_program(repeat, mode, nchunk, impl)
    return _CACHED[key]


# ----------------------------------------------------------------------------
# public entry point
# ----------------------------------------------------------------------------

def _prep_inputs(x, idx0, idx1, idx2, idx3, nchunk=NCHUNK, impl=IMPL):
    x = np.ascontiguousarray(np.asarray(x, dtype=np.float32))
    stream = _compose_stream(
        np.asarray(idx0), np.asarray(idx1), np.asarray(idx2), np.asarray(idx3),
        nchunk)
    ws = _wrap_idx(stream, np.uint16 if impl == "ic" else np.int16)
    ident = np.ascontiguousarray(np.eye(128, dtype=np.float32))
    in_maps = []
    for c in range(NCORES):
        xs = np.ascontiguousarray(x[:, c * BSLICE:(c + 1) * BSLICE].T)
        in_maps.append({"xT": xs, "si": ws, "ident": ident})
    return in_maps


def _unshard(res):
    outs = []
    for c in range(NCORES):
        o = res.results[c]["out"]                 # [128, 32, 128] tile-major
        outs.append(o.transpose(1, 0, 2).reshape(NOUT, BSLICE))
    return np.concatenate(outs, axis=1)


def kernel(x, idx0, idx1, idx2, idx3, _repeat=1):
    from concourse.bass_utils import run_bass_kernel_spmd

    in_maps = _prep_inputs(x, idx0, idx1, idx2, idx3)
    nc = _get_program(_repeat)
    res = run_bass_kernel_spmd(nc, in_maps, core_ids=list(range(NCORES)))
    return _unshard(res)


# revision 16
# speedup vs baseline: 2.1470x; 1.2823x over previous
"""Trainium2 Bass kernel for the 4-layer sum/product circuit
(nn_KnowledgeLayer): h = enc(x); h = h[idx0].prod(1); h = h[idx1].sum(1);
h = h[idx2].prod(1); h = h[idx3].sum(1).

Strategy (per core, batch sharded 8 x 128 columns, batch on PARTITIONS):
  * Host composes the four index maps into ONE fully-expanded gather from
    the 4098-entry enc table (enc = [x | 1-x | 0 | 1]).  Leaves are
    ordered "fold-halves" per chunk: the two operands of every reduction
    sit exactly half-a-chunk apart, so every circuit layer is a single
    CONTIGUOUS DVE op: h0 = g[:, :n]*g[:, n:], h1 = h0[:, :n/2]+h0[:, n/2:], ...
  * Device keeps encT as [128 batch, 4098] in SBUF and gathers along the
    FREE axis with gpsimd ap_gather (8 Q7 cores x 16 partitions each) —
    no per-row DMA descriptors at all.  ~23ns/index + ~10us/call is the
    cayman RD_CMD floor (102 cyc / 4 idx, ReadOverlap=0).
  * PE transposes h3 back to [rows, batch] via identity matmul, ACT
    drains PSUM to SBUF, DMA writes a tile-major [128, 32, 128] DRAM
    layout (per-partition contiguous => fat descriptors); the host
    unshard step permutes tiles back to row-major.

The bass program is identical for all 8 cores (pure SPMD); per-core data
(xT batch slice) is supplied via in_maps.
"""

import numpy as np

N_VARS = 2048
BATCH = 1024
NCORES = 8
BSLICE = BATCH // NCORES          # 128
TABLE = 2 * N_VARS + 2            # 4098
NIDX = 32768                      # leaves per operand stream
NOUT = 4096

NCHUNK = 8                        # default chunking of the main loop
IMPL = "apg"                      # ap_gather ucode (indirect_copy hits ISA limits and is 3-idx/req anyway)


# ----------------------------------------------------------------------------
# host-side index preparation
# ----------------------------------------------------------------------------

def _remap(e):
    """reference enc row -> our encT column."""
    e = e.astype(np.int64)
    out = np.empty_like(e)
    out[e == 0] = 2 * N_VARS
    out[e == 1] = 2 * N_VARS + 1
    even = (e >= 2) & (e % 2 == 0)
    out[even] = (e[even] - 2) // 2
    odd = (e >= 3) & (e % 2 == 1)
    out[odd] = N_VARS + (e[odd] - 3) // 2
    return out


def _compose_streams(idx0, idx1, idx2, idx3, nchunk):
    """Dedup at h2: evaluate each DISTINCT referenced h2 node once
    (fold-halves expanded subtrees, 8 leaves/node), then a second gather
    pulls h2 values for the final sum layer.

    Returns (leaf_stream, h3_stream, d2p): leaf_stream has 8*d2p indices
    into the 4098-entry enc table; h3_stream has 2*NOUT indices into the
    [d2p]-entry h2 buffer, ordered [first-operands | second-operands]."""
    uniq, inv = np.unique(idx3.reshape(-1), return_inverse=True)
    d2 = len(uniq)
    chunk_nodes = -(-d2 // (16 * nchunk)) * 16          # %16 per chunk
    d2p = chunk_nodes * nchunk
    nodes = np.concatenate([uniq, np.zeros(d2p - d2, dtype=uniq.dtype)])
    parts = []
    for c in range(nchunk):
        n2 = nodes[c * chunk_nodes:(c + 1) * chunk_nodes]
        n1 = np.concatenate([idx2[n2, 0], idx2[n2, 1]])
        n0 = np.concatenate([idx1[n1, 0], idx1[n1, 1]])
        lv = np.concatenate([idx0[n0, 0], idx0[n0, 1]])
        parts.append(_remap(lv))
    leaf_stream = np.concatenate(parts)                 # [8 * d2p]
    inv2 = inv.reshape(NOUT, 2)
    h3_stream = np.concatenate([inv2[:, 0], inv2[:, 1]])  # [2 * NOUT]
    return leaf_stream, h3_stream, d2p


def _wrap_idx(stream_idx, dtype):
    """Pool-engine index layout: stream elem k lives at partition k%16,
    column k//16, replicated across the 8 groups of 16 partitions."""
    w = stream_idx.reshape(-1, 16).T.astype(dtype)      # [16, len/16]
    return np.ascontiguousarray(np.tile(w, (8, 1)))     # [128, len/16]


# ----------------------------------------------------------------------------
# bass program (built once per config, cached)
# ----------------------------------------------------------------------------

_CACHED = {}


def _build_program(repeat=1, nchunk=NCHUNK, d2p=None):
    import contextlib
    import concourse.bacc as bacc
    import concourse.mybir as mybir
    from concourse.tile import TileContext

    f32 = mybir.dt.float32
    i16 = mybir.dt.int16
    cn = d2p // nchunk                # distinct h2 nodes per chunk
    ch = 8 * cn                       # leaves per chunk

    nc = bacc.Bacc("TRN2", target_bir_lowering=False, debug=False)

    xT = nc.dram_tensor("xT", [BSLICE, N_VARS], f32, kind="ExternalInput")
    sl = nc.dram_tensor("sl", [128, 8 * d2p // 16], i16, kind="ExternalInput")
    sq = nc.dram_tensor("sq", [128, 2 * NOUT // 16], i16, kind="ExternalInput")
    ident = nc.dram_tensor("ident", [128, 128], f32, kind="ExternalInput")
    # tile-major output: [partition, tile, batch]; host permutes to row-major
    out = nc.dram_tensor("out", [128, NOUT // 128, 128], f32,
                         kind="ExternalOutput")

    with TileContext(nc) as tc:
        with tc.tile_pool(name="setup", bufs=1) as sp, \
             tc.tile_pool(name="gather", bufs=2) as gp, \
             tc.tile_pool(name="stage", bufs=3) as stp, \
             tc.tile_pool(name="psum", bufs=2, space="PSUM") as pp:

            # ---- build encT = [x | 1-x | 0 | 1] on batch partitions ----
            encT = sp.tile([128, TABLE], f32, tag="encT")
            nc.sync.dma_start(out=encT[:, 0:N_VARS], in_=xT[:, :])
            nc.vector.tensor_scalar(
                out=encT[:, N_VARS:2 * N_VARS], in0=encT[:, 0:N_VARS],
                scalar1=-1.0, scalar2=1.0,
                op0=mybir.AluOpType.mult, op1=mybir.AluOpType.add)
            nc.vector.memset(encT[:, 2 * N_VARS:2 * N_VARS + 1], 0.0)
            nc.vector.memset(encT[:, 2 * N_VARS + 1:2 * N_VARS + 2], 1.0)

            # ---- index streams + transpose identity ----
            idL = sp.tile([128, 8 * d2p // 16], i16, tag="idL")
            for c in range(nchunk):
                csl = slice(c * (ch // 16), (c + 1) * (ch // 16))
                nc.sync.dma_start(out=idL[:, csl], in_=sl[:, csl])
            idQ = sp.tile([128, 2 * NOUT // 16], i16, tag="idQ")
            nc.sync.dma_start(out=idQ[:, :], in_=sq[:, :])
            idn = sp.tile([128, 128], f32, tag="idn")
            nc.sync.dma_start(out=idn[:, :], in_=ident[:, :])

            h2d = sp.tile([128, d2p], f32, tag="h2d")   # distinct h2 values

            loop_ctx = tc.For_i(0, repeat, 1) if repeat > 1 else contextlib.nullcontext()
            with loop_ctx:
                # ---- stage 1: evaluate distinct h2 nodes chunk by chunk ----
                for c in range(nchunk):
                    g = gp.tile([128, ch], f32, tag="g")
                    cols = slice(c * (ch // 16), (c + 1) * (ch // 16))
                    nc.gpsimd.ap_gather(
                        out_ap=g[:, :], in_ap=encT[:, :],
                        idxs_ap=idL[:, cols],
                        channels=128, num_elems=TABLE, d=1, num_idxs=ch)
                    # fold-halves, in place; final product lands in h2d slice
                    nc.vector.tensor_mul(
                        g[:, 0:ch // 2], g[:, 0:ch // 2], g[:, ch // 2:ch])
                    nc.vector.tensor_add(
                        g[:, 0:ch // 4], g[:, 0:ch // 4], g[:, ch // 4:ch // 2])
                    nc.vector.tensor_mul(
                        h2d[:, c * cn:(c + 1) * cn], g[:, 0:cn], g[:, cn:2 * cn])

                # ---- stage 2: final sum layer from deduped h2 values ----
                gq = gp.tile([128, 2 * NOUT], f32, tag="gq")
                nc.gpsimd.ap_gather(
                    out_ap=gq[:, :], in_ap=h2d[:, :],
                    idxs_ap=idQ[:, :],
                    channels=128, num_elems=d2p, d=1, num_idxs=2 * NOUT)
                nc.vector.tensor_add(
                    gq[:, 0:NOUT], gq[:, 0:NOUT], gq[:, NOUT:2 * NOUT])

                # ---- transpose h3 [128 batch, 4096 rows] -> out tiles ----
                for b in range(8):
                    ps = pp.tile([128, 4, 128], f32, tag="ps")
                    for t in range(4):
                        col = (4 * b + t) * 128
                        nc.tensor.transpose(
                            ps[:, t, :], gq[:, col:col + 128], idn[:, :])
                    st = stp.tile([128, 4, 128], f32, tag="st")
                    nc.scalar.copy(out=st[:, :, :], in_=ps[:, :, :])
                    nc.sync.dma_start(
                        out=out[:, 4 * b:4 * b + 4, :], in_=st[:, :, :])

    nc.compile()
    return nc


def _get_program(repeat=1, nchunk=NCHUNK, d2p=None):
    key = ("nc", repeat, nchunk, d2p)
    if key not in _CACHED:
        _CACHED[key] = _build_program(repeat, nchunk, d2p)
    return _CACHED[key]


# ----------------------------------------------------------------------------
# public entry point
# ----------------------------------------------------------------------------

def _prep_inputs(x, idx0, idx1, idx2, idx3, nchunk=NCHUNK):
    x = np.ascontiguousarray(np.asarray(x, dtype=np.float32))
    leaf, h3s, d2p = _compose_streams(
        np.asarray(idx0), np.asarray(idx1), np.asarray(idx2), np.asarray(idx3),
        nchunk)
    wl = _wrap_idx(leaf, np.int16)
    wq = _wrap_idx(h3s, np.int16)
    ident = np.ascontiguousarray(np.eye(128, dtype=np.float32))
    in_maps = []
    for c in range(NCORES):
        xs = np.ascontiguousarray(x[:, c * BSLICE:(c + 1) * BSLICE].T)
        in_maps.append({"xT": xs, "sl": wl, "sq": wq, "ident": ident})
    return in_maps, d2p


def _unshard(res):
    outs = []
    for c in range(NCORES):
        o = res.results[c]["out"]                 # [128, 32, 128] tile-major
        outs.append(o.transpose(1, 0, 2).reshape(NOUT, BSLICE))
    return np.concatenate(outs, axis=1)


def kernel(x, idx0, idx1, idx2, idx3, _repeat=1):
    from concourse.bass_utils import run_bass_kernel_spmd

    in_maps, d2p = _prep_inputs(x, idx0, idx1, idx2, idx3)
    nc = _get_program(_repeat, NCHUNK, d2p)
    res = run_bass_kernel_spmd(nc, in_maps, core_ids=list(range(NCORES)))
    return _unshard(res)
